# revision 5
# baseline (speedup 1.0000x reference)
"""AbstractBlast v6: packed stage-2 on PE + plain-slice DMA partition shuffles.

Stage 2 (z_o = sum_j diag(S[o,j]) y_j) reformulated as 16 dense 128x128
matmuls over rank-chunks c: Yp_c[q=(rh*16+j), t] = y_j[c*8+rh, t] gathers
all 16 j-blocks' 8-rank slices into one 128-partition tile; host-built
W2_c[q, f=(o*8+rh')] = S[o,j,c*8+rh]*(rh==rh') makes W2_c^T @ Yp_c compute
z for ALL 16 o-blocks at once. Partition orders are chosen so both layout
shuffles are expressible as single plain-slice SBUF->SBUF DMAs:
  shuffle1(c): in = Y[c*8:(c+1)*8, :, :]  (8 part, 16 j, T) -> Yp_c[128, T]
  shuffle2(o): in = Zp[o*8:(o+1)*8, :, :] (8 part, 16 c, T) -> z_o[128, T]
with z_o partition p = rh*16+c holding rank r = c*8+rh (U rows host-
permuted to match). Replaces v5's 115us DVE STT chains + 68us PE diag
matmuls with ~8us PE + ~8MB on-chip DMA.

x/Vt/Y/W2/U/z in bf16 (PSUM fp32), out fp32.
"""

import ml_dtypes
import numpy as np

import concourse.bass as bass
import concourse.mybir as mybir
from concourse.bass_utils import run_bass_kernel_spmd
from concourse.tile import TileContext

F32 = mybir.dt.float32
BF16 = mybir.dt.bfloat16

B, T, D = 8, 1024, 4096
BIN, BOUT, BSIN, BSOUT, RANK = 16, 16, 256, 256, 128
NBLK = 2
NTOK = T // NBLK  # 512 = one PSUM bank of fp32

_CACHE = {}


def _split_multi_waits(nc):
    n_split = 0
    for fn in nc.m.functions:
        for bb in fn.blocks:
            new_insts = []
            for inst in bb.instructions:
                si = inst.sync_info
                if si is not None and si.on_wait and len(si.on_wait) > 1:
                    waits = list(si.on_wait)
                    for w in waits[:-1]:
                        nop = mybir.InstNoOp(
                            name=f"{inst.name}-wsplit-{n_split}",
                            ins=[],
                            outs=[],
                            engine=inst.engine,
                            sync_info=mybir.SyncInfo(on_wait=[w], on_update=[]),
                        )
                        n_split += 1
                        new_insts.append(nop)
                    inst.sync_info = mybir.SyncInfo(
                        on_wait=[waits[-1]], on_update=list(si.on_update)
                    )
                new_insts.append(inst)
            bb.instructions = new_insts
    return n_split


def _build_kernel(split_waits=True):
    nc = bass.Bass(trn_type="TRN2")
    xt = nc.dram_tensor("xt", [BIN, 128, 2, T], BF16, kind="ExternalInput")
    vt_w = nc.dram_tensor("vt_w", [128, BIN, 2, RANK], BF16, kind="ExternalInput")
    u_w = nc.dram_tensor("u_w", [128, BOUT, BSOUT], BF16, kind="ExternalInput")
    w2_w = nc.dram_tensor("w2_w", [128, 16, 128], BF16, kind="ExternalInput")
    bias_w = nc.dram_tensor("bias_w", [128, BOUT * 2], F32, kind="ExternalInput")
    ot = nc.dram_tensor("ot", [BOUT, 128, 2, T], F32, kind="ExternalOutput")
    add = mybir.AluOpType.add

    with TileContext(nc) as tc:
        with (
            tc.tile_pool(name="wpool", bufs=1) as wpool,
            tc.tile_pool(name="xpool", bufs=4) as xpool,
            tc.tile_pool(name="ypool", bufs=1) as ypool,
            tc.tile_pool(name="yppool", bufs=1) as yppool,
            tc.tile_pool(name="zpool", bufs=4) as zpool,
            tc.tile_pool(name="opool", bufs=3) as opool,
            tc.tile_pool(name="ypsum", bufs=2, space="PSUM") as ypsum,
            tc.tile_pool(name="zpsum", bufs=2, space="PSUM") as zpsum,
            tc.tile_pool(name="opsum", bufs=4, space="PSUM") as opsum,
        ):
            vt_t = wpool.tile([128, BIN, 2, RANK], BF16)
            nc.sync.dma_start(out=vt_t, in_=vt_w[:, :, :, :])
            u_t = wpool.tile([128, BOUT, BSOUT], BF16)
            nc.sync.dma_start(out=u_t, in_=u_w[:, :, :])
            w2_t = wpool.tile([128, 16, 128], BF16)
            nc.sync.dma_start(out=w2_t, in_=w2_w[:, :, :])
            bias_t = wpool.tile([128, BOUT * 2], F32)
            nc.sync.dma_start(out=bias_t, in_=bias_w[:, :])

            # Y: stage-1 output, partition = rank, free = (j, t)
            y_t = ypool.tile([128, BIN, T], BF16, tag="y")
            # Zp: stage-2 output, partition = (o*8+rh'), free = (c, t)
            zp_t = ypool.tile([128, 16, T], BF16, tag="zp")

            # ---- stage 1: y_j = Vt_j^T x_j ----
            for j in range(BIN):
                x_t = xpool.tile([128, 2, T], BF16, tag="xt")
                nc.sync.dma_start(out=x_t, in_=xt[j, :, :, :])
                for blk in range(NBLK):
                    tok = slice(blk * NTOK, (blk + 1) * NTOK)
                    y_ps = ypsum.tile([128, NTOK], F32)
                    for k in range(2):
                        nc.tensor.matmul(
                            y_ps,
                            vt_t[:, j, k, :],
                            x_t[:, k, tok],
                            start=(k == 0),
                            stop=(k == 1),
                        )
                    if (j * NBLK + blk) % 2 == 0:
                        nc.vector.tensor_copy(y_t[:, j, tok], y_ps)
                    else:
                        nc.scalar.copy(y_t[:, j, tok], y_ps)

            # ---- stage 2: shuffle1 + Z'_c = W2_c^T Yp_c (all o at once) ----
            for c in range(16):
                yp_c = yppool.tile([128, T], BF16, tag=f"yp{c}")
                nc.sync.dma_start(out=yp_c, in_=y_t[c * 8 : (c + 1) * 8, :, :])
                for blk in range(NBLK):
                    tok = slice(blk * NTOK, (blk + 1) * NTOK)
                    z_ps = zpsum.tile([128, NTOK], F32)
                    nc.tensor.matmul(
                        z_ps,
                        w2_t[:, c, :],
                        yp_c[:, tok],
                        start=True,
                        stop=True,
                    )
                    if (c * NBLK + blk) % 2 == 0:
                        nc.vector.tensor_copy(zp_t[:, c, tok], z_ps)
                    else:
                        nc.scalar.copy(zp_t[:, c, tok], z_ps)

            # ---- stage 3: shuffle2 + out_o = U_o^T z_o + bias ----
            for o in range(BOUT):
                z = zpool.tile([128, T], BF16, tag="z")
                nc.sync.dma_start(out=z, in_=zp_t[o * 8 : (o + 1) * 8, :, :])
                o_sb = opool.tile([128, 2, T], F32, tag="o")
                for blk in range(NBLK):
                    tok = slice(blk * NTOK, (blk + 1) * NTOK)
                    for h in range(2):
                        o_ps = opsum.tile([128, NTOK], F32)
                        nc.tensor.matmul(
                            o_ps,
                            u_t[:, o, 128 * h : 128 * (h + 1)],
                            z[:, tok],
                            start=True,
                            stop=True,
                        )
                        if (blk * 2 + h) % 2 == 0:
                            nc.scalar.activation(
                                o_sb[:, h, tok],
                                o_ps,
                                mybir.ActivationFunctionType.Identity,
                                bias=bias_t[:, 2 * o + h : 2 * o + h + 1],
                                scale=1.0,
                            )
                        else:
                            nc.vector.tensor_scalar(
                                o_sb[:, h, tok],
                                o_ps,
                                bias_t[:, 2 * o + h : 2 * o + h + 1],
                                None,
                                add,
                            )
                nc.sync.dma_start(out=ot[o, :, :, :], in_=o_sb)

    if split_waits:
        _split_multi_waits(nc)
    return nc


def _prep_weights(S, U, Vt, bias):
    bf = ml_dtypes.bfloat16
    vt_w = np.ascontiguousarray(
        Vt.reshape(BIN, 2, 128, RANK).transpose(2, 0, 1, 3).astype(bf)
    )
    # u rows permuted: partition p=(rh*16+c) holds rank r=c*8+rh
    perm = np.array([(p % 16) * 8 + p // 16 for p in range(128)])
    u_w = np.ascontiguousarray(U.transpose(1, 0, 2)[perm].astype(bf))
    # W2[q=(rh*16+j), c, f=(o*8+rh')] = S[o, j, c*8+rh] if rh == rh'
    S4 = S.reshape(BOUT, BIN, 16, 8)  # [o, j, c, rh]
    w2 = np.zeros((8, BIN, 16, BOUT, 8), dtype=np.float32)  # [rh, j, c, o, rh']
    for rh in range(8):
        w2[rh, :, :, :, rh] = S4[:, :, :, rh].transpose(1, 2, 0)
    w2_w = np.ascontiguousarray(w2.reshape(128, 16, 128).astype(bf))
    bias_w = np.ascontiguousarray(
        bias.reshape(BOUT, 2, 128).transpose(2, 0, 1).reshape(128, BOUT * 2)
    )
    return vt_w, u_w, w2_w, bias_w


def kernel(x, S, U, Vt, bias):
    x = np.asarray(x, dtype=np.float32)
    S = np.asarray(S, dtype=np.float32)
    U = np.asarray(U, dtype=np.float32)
    Vt = np.asarray(Vt, dtype=np.float32)
    bias = np.asarray(bias, dtype=np.float32)

    bf = ml_dtypes.bfloat16
    vt_w, u_w, w2_w, bias_w = _prep_weights(S, U, Vt, bias)

    if "nc" not in _CACHE:
        _CACHE["nc"] = _build_kernel()
    nc = _CACHE["nc"]

    in_maps = []
    for b in range(B):
        xt = np.ascontiguousarray(
            x[b].T.reshape(BIN, 2, 128, T).transpose(0, 2, 1, 3).astype(bf)
        )
        in_maps.append(
            {"xt": xt, "vt_w": vt_w, "u_w": u_w, "w2_w": w2_w, "bias_w": bias_w}
        )

    res = run_bass_kernel_spmd(nc, in_maps, core_ids=list(range(B)))

    out = np.empty((B, T, D), dtype=np.float32)
    for b in range(B):
        o = res.results[b]["ot"]
        out[b] = o.transpose(3, 0, 2, 1).reshape(T, D)
    return out
